# revision 2
# baseline (speedup 1.0000x reference)
"""Causal self-attention (B=4, S=2048, E=1024, H=16) on 8 trn2 cores — v2.

Sharding: batch x head-octet. Core c owns batch b=c//2 and heads
g=c%2 (heads 8g..8g+7, d-slice cols 512g..512g+512):
  - computes q,k,v for its 8 heads from its batch's x,
  - runs causal attention for those heads,
  - multiplies by its 512-row slice of W_proj producing a PARTIAL [S, E]
    output; the host sums each batch's 2 partials and adds b_proj.

Engine balance (single batch per core, ~490k PE cycles):
  - PE: qkv (pumped into the attention phases as filler), scores [k,q],
    flipped w@V (out [128q, 65] per k-chunk: full M=128 vs 65 in the
    [65,w] orientation), V/y transposes, projection.
  - ACT: exp only, batched over chunk-PAIRS ([128,2,512] psum -> bf16),
    plus the tail output copies.
  - DVE: qkv bias add (psum->SBUF, q pre-scaled 1/8 host-side), causal
    mask multiply, reciprocal + normalize, psum->SBUF transpose copies.
  - Pool/GPSIMD cannot touch PSUM on HW; it only builds constants.
  - PSUM accumulation state is per-BANK: each y tile forms ONE
    start/stop group across all its q-subtile regions.
  - The attention inner loop is a flat lag-1 software pipeline over
    (head, chunk-pair) units, including across head boundaries, with a
    cost-weighted pump of qkv/projection filler between units.
  - All projection + y-transposes are DEFERRED to the last attention
    phase (otherwise ACT-exp-bound, and qkv filler cannot exist there).

dtypes: fp16 for x/W/QT/KT/yn/yT/Wp; bf16 for exp(scores) and V (exp of
unnormalized scores needs bf16 range). PSUM f32; output partial bf16,
summed in f32 on host.
"""

import sys

if "/opt/trn_rl_repo" not in sys.path:
    sys.path.insert(0, "/opt/trn_rl_repo")

import numpy as np

N_EMBD = 1024
N_HEAD = 16
D = 64
N_CORES = 8
B_FULL = 4
S_FULL = 2048
HPC = 8  # heads per core


def _patch_tile(tile):
    """This container's walrus build allows max 1 sem wait per instruction;
    stock Tile can attach several (tail drain, and any instruction whose
    inputs come from 2+ engines/queues). Split extras onto standalone
    single-wait nop carriers on the same engine, emitted just before."""
    if getattr(tile.TileContext, "_drain_split_patched", False):
        return

    orig_commit = tile.TileContext._commit_instruction

    def _commit_instruction(self, inst, lazy_reg_writes=True):
        si = inst.sync_info
        waits = list(si.on_wait) if si is not None and si.on_wait else []
        if len(waits) > 1:
            by_name = {h.name: h for h in self.sems.allocated().values()}
            for w in waits[:-1]:
                h = by_name.get(w.ant_name)
                if h is None:
                    raise RuntimeError(f"wait-split: no handle for {w.ant_name}")
                nop = self.nc.engines[inst.engine].nop(nofuse=True)
                nop.wait_op(h, w.wait_value, _wait_mode_op(w), check=False)
            inst.sync_info.on_wait = [waits[-1]]
        return orig_commit(self, inst, lazy_reg_writes)

    def _wait_mode_op(w):
        m = str(w.wait_mode)
        if "ge" in m:
            return "sem-ge"
        if "eq" in m:
            return "sem-eq"
        raise RuntimeError(f"wait-split: unsupported wait mode {m}")

    tile.TileContext._commit_instruction = _commit_instruction

    def _drain_and_barrier(self, tick_clock, wait_clock):
        nc = self.nc
        drain_inst = nc.sync.drain()
        wait_clock.add_sem_waits(
            drain_inst.ins, tile.ScopedClock({None: tick_clock.global_clock})
        )
        waits = list(drain_inst.ins.sync_info.on_wait or [])
        if len(waits) > 1:
            drain_inst.ins.sync_info.on_wait = [waits[0]]
            by_name = {}
            if self.sems is not None:
                by_name = {h.name: h for h in self.sems.allocated().values()}
            for w in waits[1:]:
                extra = nc.sync.drain()
                h = by_name.get(w.ant_name)
                if h is None:
                    raise RuntimeError(f"drain-split: no handle for {w.ant_name}")
                extra._wait_ge(h, w.wait_value)
        nc.all_engine_barrier()
        assert self.sems is not None
        popped = nc._tile_sem_poison_stack.pop()
        assert popped is self._sem_poison
        nc.clear_and_free_semaphores(list(self.sems.allocated().values()))
        nc.all_engine_barrier()

    tile.TileContext._drain_and_barrier = _drain_and_barrier
    tile.TileContext._drain_split_patched = True


def build_nc(s=S_FULL, num_devices=N_CORES):
    import concourse.bass as bass
    import concourse.mybir as mybir
    import concourse.tile as tile
    from concourse.bass import ds, ts
    from concourse.masks import make_identity

    _patch_tile(tile)

    f32 = mybir.dt.float32
    f16 = mybir.dt.float16
    bf16 = mybir.dt.bfloat16
    AF = mybir.ActivationFunctionType
    ALU = mybir.AluOpType
    E = N_EMBD
    KO = E // 128     # contraction chunks for qkv (8)
    NT = s // 512     # 512-token tiles (4)
    NKC = s // 128    # 128-token k chunks (16)
    NQS = s // 128    # 128-token q subtiles (16)
    NCG = 12          # qkv col groups of 128 (4 q, 4 k, 4 v)

    nc = bass.Bass(
        "TRN2", target_bir_lowering=False, debug=False, num_devices=num_devices
    )
    xT = nc.dram_tensor("xT", [E, s], f16, kind="ExternalInput")
    Wqkv = nc.dram_tensor("Wqkv", [E, NCG * 128], f16, kind="ExternalInput")
    bqkv = nc.dram_tensor("bqkv", [128, NCG], f32, kind="ExternalInput")
    Wp = nc.dram_tensor("Wp", [HPC * D, E], f16, kind="ExternalInput")
    out = nc.dram_tensor("out", [s, E], bf16, kind="ExternalOutput")

    xT_r = xT[:].rearrange("(ko p) t -> p ko t", p=128)
    W_r = Wqkv[:].rearrange("(ko p) c -> p ko c", p=128)
    Wp_r = Wp[:].rearrange("(dg p) e -> p dg e", p=128)

    from contextlib import ExitStack

    with tile.TileContext(nc) as tc, ExitStack() as ctx:
        const = ctx.enter_context(tc.tile_pool(name="const", bufs=1))
        xp = ctx.enter_context(tc.tile_pool(name="xp", bufs=2))
        qtp = ctx.enter_context(tc.tile_pool(name="qtp", bufs=2))
        vtp = ctx.enter_context(tc.tile_pool(name="vtp", bufs=2))
        expp = ctx.enter_context(tc.tile_pool(name="expp", bufs=8))
        ytp = ctx.enter_context(tc.tile_pool(name="ytp", bufs=2))
        recp = ctx.enter_context(tc.tile_pool(name="recp", bufs=2))
        obp = ctx.enter_context(tc.tile_pool(name="obp", bufs=3))
        sp_ps = ctx.enter_context(tc.tile_pool(name="spps", bufs=2, space="PSUM"))
        y_ps = ctx.enter_context(tc.tile_pool(name="yps", bufs=2, space="PSUM"))
        aux_ps = ctx.enter_context(tc.tile_pool(name="auxps", bufs=2, space="PSUM"))

        # ---- constants / persistent ----
        Wsb = const.tile([128, KO, NCG * 128], f16, tag="w")
        bsb = const.tile([128, NCG], f32, tag="b")
        Wp_sb = const.tile([128, 4, E], f16, tag="wp")
        KT = const.tile([128, 4, s], f16, tag="kt")
        Vaug = const.tile([128, NKC, 4, 2, D + 1], bf16, tag="vaug")
        yns = [
            const.tile([128, 4, HPC, D], f16, tag=f"yn{q}", name=f"yn{q}")
            for q in range(NT)
        ]
        # yT staging for the last q-tile's subtiles (filled per head-pair as
        # the final phase's heads complete, shrinking the serial tail)
        yTtail = const.tile([128, 4, 4, 128], f16, tag="yTtail")
        ident_f32 = const.tile([128, 128], f32, tag="id32")
        ident16 = const.tile([128, 128], f16, tag="id16")
        ident_bf = const.tile([128, 128], bf16, tag="idbf")
        ones = const.tile([128, NKC, 4, 2], f32, tag="ones")

        make_identity(nc, ident_f32[:])
        nc.vector.tensor_copy(ident16[:], ident_f32[:])
        nc.vector.tensor_copy(ident_bf[:], ident_f32[:])
        nc.gpsimd.memset(ones[:], 1.0)
        # Vaug ones column (softmax denominator via the extra matmul column)
        nc.vector.tensor_copy(Vaug[:, :, :, :, D], ones[:])
        # causal mask: with per-chunk 128*m narrowing, ONLY the chunk's own
        # diagonal q-subtile (cols [0:128] of each channel) needs masking —
        # a single [128,2,128] lower-triangle multiplier (DVE, 2-byte fast)
        maskD = const.tile([128, 2, 128], bf16, tag="maskD")
        nc.gpsimd.memset(maskD[:], 1.0)
        nc.gpsimd.affine_select(
            maskD[:],
            maskD[:],
            pattern=[[0, 2], [1, 128]],
            compare_op=ALU.is_ge,
            fill=0.0,
            base=0,
            channel_multiplier=-1,
        )

        tiles = {}

        def qkv_steps(nt):
            """Emit qkv + V-transpose for token-512-tile nt in small steps so
            the caller can interleave them into the attention stream."""
            xt = xp.tile([128, KO, 512], f16, tag="xt")
            if nt == 0:
                # first x half-tile on SP; bias + weights in parallel on the
                # ACT hwdge queue (ACT is idle through the prologue); bias
                # first — the first Pool bias op gates the qkv PSUM rotation
                nc.scalar.dma_start(bsb[:], bqkv[:])
                nc.scalar.dma_start(
                    Wsb[:, 0:4, ds(0, 128)], W_r[:, 0:4, ds(0, 128)]
                )
                nc.sync.dma_start(
                    xt[:, 0:2, :], xT_r[:, 0:2, ds(nt * 512, 512)]
                )
                nc.scalar.dma_start(
                    Wsb[:, 4:KO, ds(0, 128)], W_r[:, 4:KO, ds(0, 128)]
                )
                nc.sync.dma_start(
                    xt[:, 2:4, :], xT_r[:, 2:4, ds(nt * 512, 512)]
                )
                nc.scalar.dma_start(
                    Wsb[:, :, ds(128, 128)], W_r[:, :, ds(128, 128)]
                )
                nc.sync.dma_start(
                    xt[:, 4:6, :], xT_r[:, 4:6, ds(nt * 512, 512)]
                )
                nc.scalar.dma_start(
                    Wsb[:, :, ds(256, 128)], W_r[:, :, ds(256, 128)]
                )
                nc.sync.dma_start(
                    xt[:, 6:KO, :], xT_r[:, 6:KO, ds(nt * 512, 512)]
                )
                for cg in range(3, NCG):
                    nc.scalar.dma_start(
                        Wsb[:, :, ds(cg * 128, 128)],
                        W_r[:, :, ds(cg * 128, 128)],
                    )
            else:
                nc.sync.dma_start(xt[:], xT_r[:, :, ds(nt * 512, 512)])
                if nt == 1:
                    nc.scalar.dma_start(Wp_sb[:], Wp_r[:])
            QTn = qtp.tile([128, 4, 512], f16, tag="qt", name=f"QT{nt}")
            VTn = vtp.tile([128, 4, 512], bf16, tag="vt", name=f"VT{nt}")
            tiles[nt] = QTn
            yield 0
            for cg in range(NCG):
                kind, g = cg // 4, cg % 4
                ps = aux_ps.tile([128, 512], f32, tag="aux")
                for ko in range(KO):
                    nc.tensor.matmul(
                        ps[:],
                        lhsT=Wsb[:, ko, ds(cg * 128, 128)],
                        rhs=xt[:, ko],
                        start=(ko == 0),
                        stop=(ko == KO - 1),
                    )
                    if ko % 2 == 1:
                        yield 427
                bias = bsb[:, ds(cg, 1)]
                if kind == 0:
                    dst = QTn[:, g, :]  # q pre-scaled by 1/8 host-side
                elif kind == 1:
                    dst = KT[:, g, ds(nt * 512, 512)]
                else:
                    dst = VTn[:, g, :]
                nc.vector.tensor_scalar_add(dst, ps[:], bias)
                yield 0
                if kind == 2:
                    # V token-major: one [128,128] transpose covers both
                    # heads of group g for each token-128 chunk
                    for tt in range(4):
                        kcg = nt * 4 + tt
                        tp = aux_ps.tile([128, 2, D], bf16, tag="aux")
                        nc.tensor.transpose(
                            tp[:], VTn[:, g, ds(tt * 128, 128)], ident_bf[:]
                        )
                        nc.vector.tensor_copy(Vaug[:, kcg, g, :, 0:D], tp[:])
                        yield 60

        def proj_steps(js_list):
            """y^T via PE transpose + projection partial for each global
            128-q-subtile in js_list."""
            for js in js_list:
                yTt = ytp.tile([128, 4, 128], f16, tag="yT")
                for hp in range(4):
                    tp = aux_ps.tile([128, 128], f16, tag="aux")
                    nc.tensor.transpose(
                        tp[:], yns[js // 4][:, js % 4, ds(2 * hp, 2), :],
                        ident16[:],
                    )
                    nc.vector.tensor_copy(yTt[:, hp, :], tp[:])
                    yield 60
                for n in range(2):
                    po = aux_ps.tile([128, 512], f32, tag="aux")
                    for dg in range(4):
                        nc.tensor.matmul(
                            po[:],
                            lhsT=yTt[:, dg, :],
                            rhs=Wp_sb[:, dg, ds(n * 512, 512)],
                            start=(dg == 0),
                            stop=(dg == 3),
                        )
                    ob = obp.tile([128, 512], bf16, tag="ob")
                    nc.vector.tensor_copy(ob[:], po[:])
                    nc.sync.dma_start(
                        out[ds(js * 128, 128), ds(n * 512, 512)], ob[:]
                    )
                    yield 853

        def emit_wv(qt, h, prev, yt):
            # PSUM accumulation state is per-BANK (one written-bitmap): the
            # whole tile must form ONE group — start only on the very first
            # matmul, stop on the very last; first write per address within
            # the group replaces, later ones accumulate.
            p, et = prev
            for c in range(2):
                kc = 2 * p + c
                m = kc - 4 * qt
                offc = max(0, 128 * m)
                for j in range(4):
                    if j < m:
                        continue
                    nc.tensor.matmul(
                        yt[:, j, :],
                        lhsT=et[:, c, ds(j * 128 - offc, 128)],
                        rhs=Vaug[:, kc, h // 2, h % 2, :],
                        start=(kc == 0 and j == 0),
                        stop=(kc == 4 * qt + 3 and j == 3),
                        skip_group_check=True,
                    )

        gen0 = qkv_steps(0)
        for _ in gen0:
            pass

        for qt in range(NT):
            QTn = tiles[qt]
            npairs = 2 * qt + 2
            # pump source: next tile's qkv during phases 0-2 (plus the first
            # projection subtiles late in phase 2); remaining projection
            # during the last (exp-heavy) phase. Rate-paced so the PE filler
            # spreads across the whole phase instead of draining early.
            slots = HPC * (2 * npairs + 2)
            if qt + 1 < NT:
                pumps = [qkv_steps(qt + 1)]
                total_ns = 22000.0
            else:
                pumps = [proj_steps(range(0, 12))]
                total_ns = 23100.0
            budget_ns = total_ns / slots
            acc = [0.0]

            def pump(n):
                acc[0] += n * budget_ns
                while acc[0] > 0.0 and pumps:
                    step = next(pumps[0], None)
                    if step is None:
                        pumps.pop(0)
                    else:
                        acc[0] -= max(step, 60.0)

            def finish_head(h, yt):
                """recip + normalize for head h (after its last wv), plus the
                last q-tile's eager per-head-pair transposes."""
                rec = recp.tile([128, 4], f32, tag="rec")
                nc.vector.reciprocal(rec[:], yt[:, :, D])
                for j in range(4):
                    nc.vector.tensor_scalar_mul(
                        yns[qt][:, j, h, :], yt[:, j, 0:D], rec[:, ds(j, 1)]
                    )
                if qt == NT - 1 and h % 2 == 1:
                    hpd = h // 2
                    for jsi in range(4):
                        tp = aux_ps.tile([128, 128], f16, tag="aux")
                        nc.tensor.transpose(
                            tp[:], yns[qt][:, jsi, ds(2 * hpd, 2), :],
                            ident16[:],
                        )
                        nc.vector.tensor_copy(yTtail[:, jsi, hpd, :], tp[:])

            # flat lag-1 pipeline over (head, chunk-pair) units: the wv of
            # unit i-1 is emitted after unit i's scores+exp, ALSO across head
            # boundaries, so no wv ever waits zero-lag on its own exp
            yts = {}
            prev = None
            for h in range(HPC):
                g2, hp = h % 2, h // 2
                for p in range(npairs):
                    m0 = 2 * p - 4 * qt
                    off = max(0, 128 * m0)
                    w = 512 - off
                    sp = sp_ps.tile([128, 2, 512], f32, tag="s")
                    for c in range(2):
                        kc = 2 * p + c
                        offc = max(0, 128 * (m0 + c))
                        nc.tensor.matmul(
                            sp[:, c, 0 : 512 - offc],
                            lhsT=KT[ds(64 * g2, 64), hp, ds(kc * 128, 128)],
                            rhs=QTn[ds(64 * g2, 64), hp, ds(offc, 512 - offc)],
                            start=True,
                            stop=True,
                        )
                    et = expp.tile([128, 2, 512], bf16, tag="exp")
                    nc.scalar.activation(et[:, :, 0:w], sp[:, :, 0:w], AF.Exp)
                    if m0 >= 0:
                        # causal: each chunk's own-diagonal q-subtile sits at
                        # cols [0:128] of its channel after narrowing
                        nc.vector.tensor_mul(
                            et[:, :, 0:128], et[:, :, 0:128], maskD[:]
                        )
                    pump(1)
                    if prev is not None:
                        ph, pp, pet = prev
                        if pp == 0:
                            yts[ph] = y_ps.tile(
                                [128, 4, D + 1], f32, tag="y",
                                name=f"y{qt}_{ph}",
                            )
                        emit_wv(qt, ph, (pp, pet), yts[ph])
                        if pp == npairs - 1:
                            finish_head(ph, yts[ph])
                        pump(1)
                    prev = (h, p, et)
            ph, pp, pet = prev
            if pp == 0:
                yts[ph] = y_ps.tile(
                    [128, 4, D + 1], f32, tag="y", name=f"y{qt}_{ph}"
                )
            pump(2)
            emit_wv(qt, ph, (pp, pet), yts[ph])
            finish_head(ph, yts[ph])
            # drain this phase's pump sources
            while pumps:
                if next(pumps[0], "done") == "done":
                    pumps.pop(0)

        # final projection for the last q-tile's subtiles (transposes already
        # staged in yTtail by the head-pair-gated path above)
        for jsi in range(4):
            for n in range(2):
                po = aux_ps.tile([128, 512], f32, tag="aux")
                for dg in range(4):
                    nc.tensor.matmul(
                        po[:],
                        lhsT=yTtail[:, jsi, dg, :],
                        rhs=Wp_sb[:, dg, ds(n * 512, 512)],
                        start=(dg == 0),
                        stop=(dg == 3),
                    )
                ob = obp.tile([128, 512], bf16, tag="ob")
                nc.scalar.activation(ob[:], po[:], AF.Identity)
                nc.sync.dma_start(
                    out[ds((12 + jsi) * 128, 128), ds(n * 512, 512)], ob[:]
                )

    return nc


def shard_inputs(x, W_attn, b_attn, W_proj):
    """Per-core input maps: core c = 2*b + g owns batch b, heads 8g..8g+8."""
    E = N_EMBD
    nb, s, _ = x.shape
    in_maps = []
    for c in range(N_CORES):
        b, g = c // 2, c % 2
        lo = 512 * g
        Wq = W_attn[:, lo : lo + 512] * 0.125
        Wk = W_attn[:, E + lo : E + lo + 512]
        Wv = W_attn[:, 2 * E + lo : 2 * E + lo + 512]
        Wqkv = np.concatenate([Wq, Wk, Wv], axis=1).astype(np.float16)
        bq = b_attn[lo : lo + 512] * 0.125
        bk = b_attn[E + lo : E + lo + 512]
        bv = b_attn[2 * E + lo : 2 * E + lo + 512]
        bqkv = (
            np.concatenate([bq, bk, bv]).reshape(12, 128).T.astype(np.float32)
        )
        in_maps.append(
            {
                "xT": np.ascontiguousarray(x[b].T).astype(np.float16),
                "Wqkv": np.ascontiguousarray(Wqkv),
                "bqkv": np.ascontiguousarray(bqkv),
                "Wp": W_proj[lo : lo + 512, :].astype(np.float16),
            }
        )
    return in_maps


_NC_CACHE = {}


def _get_nc(s=S_FULL):
    if s not in _NC_CACHE:
        _NC_CACHE[s] = build_nc(s)
    return _NC_CACHE[s]


def kernel(x, W_attn, b_attn, W_proj, b_proj, _trace=False):
    from concourse.bass_utils import run_bass_kernel_spmd

    nb, s, E = x.shape
    assert E == N_EMBD
    nc = _get_nc(s)
    in_maps = shard_inputs(x, W_attn, b_attn, W_proj)
    res = run_bass_kernel_spmd(nc, in_maps, list(range(N_CORES)), trace=_trace)
    outs = []
    for b in range(nb):
        acc = res.results[2 * b]["out"].astype(np.float32)
        acc += res.results[2 * b + 1]["out"]
        acc += b_proj.astype(np.float32)
        outs.append(acc)
    kernel.last_results = res
    return np.stack(outs, axis=0)


# revision 4
# speedup vs baseline: 1.0008x; 1.0008x over previous
"""Causal self-attention (B=4, S=2048, E=1024, H=16) on 8 trn2 cores — v2.

Sharding: batch x head-octet. Core c owns batch b=c//2 and heads
g=c%2 (heads 8g..8g+7, d-slice cols 512g..512g+512):
  - computes q,k,v for its 8 heads from its batch's x,
  - runs causal attention for those heads,
  - multiplies by its 512-row slice of W_proj producing a PARTIAL [S, E]
    output; the host sums each batch's 2 partials and adds b_proj.

Engine balance (single batch per core, ~490k PE cycles):
  - PE: qkv (pumped into the attention phases as filler), scores [k,q],
    flipped w@V (out [128q, 65] per k-chunk: full M=128 vs 65 in the
    [65,w] orientation), V/y transposes, projection.
  - ACT: exp only, batched over chunk-PAIRS ([128,2,512] psum -> bf16),
    plus the tail output copies.
  - DVE: qkv bias add (psum->SBUF, q pre-scaled 1/8 host-side), causal
    mask multiply, reciprocal + normalize, psum->SBUF transpose copies.
  - Pool/GPSIMD cannot touch PSUM on HW; it only builds constants.
  - PSUM accumulation state is per-BANK: each y tile forms ONE
    start/stop group across all its q-subtile regions.
  - The attention inner loop is a flat lag-1 software pipeline over
    (head, chunk-pair) units, including across head boundaries, with a
    cost-weighted pump of qkv/projection filler between units.
  - All projection + y-transposes are DEFERRED to the last attention
    phase (otherwise ACT-exp-bound, and qkv filler cannot exist there).

dtypes: fp16 for x/W/QT/KT/yn/yT/Wp; bf16 for exp(scores) and V (exp of
unnormalized scores needs bf16 range). PSUM f32; output partial bf16,
summed in f32 on host.
"""

import sys

if "/opt/trn_rl_repo" not in sys.path:
    sys.path.insert(0, "/opt/trn_rl_repo")

import numpy as np

N_EMBD = 1024
N_HEAD = 16
D = 64
N_CORES = 8
B_FULL = 4
S_FULL = 2048
HPC = 8  # heads per core


def _patch_tile(tile):
    """This container's walrus build allows max 1 sem wait per instruction;
    stock Tile can attach several (tail drain, and any instruction whose
    inputs come from 2+ engines/queues). Split extras onto standalone
    single-wait nop carriers on the same engine, emitted just before."""
    if getattr(tile.TileContext, "_drain_split_patched", False):
        return

    orig_commit = tile.TileContext._commit_instruction

    def _commit_instruction(self, inst, lazy_reg_writes=True):
        si = inst.sync_info
        waits = list(si.on_wait) if si is not None and si.on_wait else []
        if len(waits) > 1:
            by_name = {h.name: h for h in self.sems.allocated().values()}
            for w in waits[:-1]:
                h = by_name.get(w.ant_name)
                if h is None:
                    raise RuntimeError(f"wait-split: no handle for {w.ant_name}")
                nop = self.nc.engines[inst.engine].nop(nofuse=True)
                nop.wait_op(h, w.wait_value, _wait_mode_op(w), check=False)
            inst.sync_info.on_wait = [waits[-1]]
        return orig_commit(self, inst, lazy_reg_writes)

    def _wait_mode_op(w):
        m = str(w.wait_mode)
        if "ge" in m:
            return "sem-ge"
        if "eq" in m:
            return "sem-eq"
        raise RuntimeError(f"wait-split: unsupported wait mode {m}")

    tile.TileContext._commit_instruction = _commit_instruction

    def _drain_and_barrier(self, tick_clock, wait_clock):
        nc = self.nc
        drain_inst = nc.sync.drain()
        wait_clock.add_sem_waits(
            drain_inst.ins, tile.ScopedClock({None: tick_clock.global_clock})
        )
        waits = list(drain_inst.ins.sync_info.on_wait or [])
        if len(waits) > 1:
            drain_inst.ins.sync_info.on_wait = [waits[0]]
            by_name = {}
            if self.sems is not None:
                by_name = {h.name: h for h in self.sems.allocated().values()}
            for w in waits[1:]:
                extra = nc.sync.drain()
                h = by_name.get(w.ant_name)
                if h is None:
                    raise RuntimeError(f"drain-split: no handle for {w.ant_name}")
                extra._wait_ge(h, w.wait_value)
        nc.all_engine_barrier()
        assert self.sems is not None
        popped = nc._tile_sem_poison_stack.pop()
        assert popped is self._sem_poison
        nc.clear_and_free_semaphores(list(self.sems.allocated().values()))
        nc.all_engine_barrier()

    tile.TileContext._drain_and_barrier = _drain_and_barrier
    tile.TileContext._drain_split_patched = True


def build_nc(s=S_FULL, num_devices=N_CORES):
    import concourse.bass as bass
    import concourse.mybir as mybir
    import concourse.tile as tile
    from concourse.bass import ds, ts
    from concourse.masks import make_identity

    _patch_tile(tile)

    f32 = mybir.dt.float32
    f16 = mybir.dt.float16
    bf16 = mybir.dt.bfloat16
    AF = mybir.ActivationFunctionType
    ALU = mybir.AluOpType
    E = N_EMBD
    KO = E // 128     # contraction chunks for qkv (8)
    NT = s // 512     # 512-token tiles (4)
    NKC = s // 128    # 128-token k chunks (16)
    NQS = s // 128    # 128-token q subtiles (16)
    NCG = 12          # qkv col groups of 128 (4 q, 4 k, 4 v)

    nc = bass.Bass(
        "TRN2", target_bir_lowering=False, debug=False, num_devices=num_devices
    )
    xT = nc.dram_tensor("xT", [E, s], f16, kind="ExternalInput")
    Wqkv = nc.dram_tensor("Wqkv", [E, NCG * 128], f16, kind="ExternalInput")
    bqkv = nc.dram_tensor("bqkv", [128, NCG], f32, kind="ExternalInput")
    Wp = nc.dram_tensor("Wp", [HPC * D, E], f16, kind="ExternalInput")
    out = nc.dram_tensor("out", [s, E], bf16, kind="ExternalOutput")

    xT_r = xT[:].rearrange("(ko p) t -> p ko t", p=128)
    W_r = Wqkv[:].rearrange("(ko p) c -> p ko c", p=128)
    Wp_r = Wp[:].rearrange("(dg p) e -> p dg e", p=128)

    from contextlib import ExitStack

    with tile.TileContext(nc) as tc, ExitStack() as ctx:
        const = ctx.enter_context(tc.tile_pool(name="const", bufs=1))
        xp = ctx.enter_context(tc.tile_pool(name="xp", bufs=2))
        qtp = ctx.enter_context(tc.tile_pool(name="qtp", bufs=2))
        vtp = ctx.enter_context(tc.tile_pool(name="vtp", bufs=2))
        expp = ctx.enter_context(tc.tile_pool(name="expp", bufs=8))
        ytp = ctx.enter_context(tc.tile_pool(name="ytp", bufs=2))
        recp = ctx.enter_context(tc.tile_pool(name="recp", bufs=2))
        obp = ctx.enter_context(tc.tile_pool(name="obp", bufs=3))
        sp_ps = ctx.enter_context(tc.tile_pool(name="spps", bufs=2, space="PSUM"))
        y_ps = ctx.enter_context(tc.tile_pool(name="yps", bufs=2, space="PSUM"))
        aux_ps = ctx.enter_context(tc.tile_pool(name="auxps", bufs=2, space="PSUM"))

        # ---- constants / persistent ----
        Wsb = const.tile([128, KO, NCG * 128], f16, tag="w")
        bsb = const.tile([128, NCG], f32, tag="b")
        Wp_sb = const.tile([128, 4, E], f16, tag="wp")
        KT = const.tile([128, 4, s], f16, tag="kt")
        Vaug = const.tile([128, NKC, 4, 2, D + 1], bf16, tag="vaug")
        yns = [
            const.tile([128, 4, HPC, D], f16, tag=f"yn{q}", name=f"yn{q}")
            for q in range(NT)
        ]
        # yT staging for the last q-tile's subtiles (filled per head-pair as
        # the final phase's heads complete, shrinking the serial tail)
        yTtail = const.tile([128, 4, 4, 128], f16, tag="yTtail")
        ident_f32 = const.tile([128, 128], f32, tag="id32")
        ident16 = const.tile([128, 128], f16, tag="id16")
        ident_bf = const.tile([128, 128], bf16, tag="idbf")
        ones = const.tile([128, NKC, 4, 2], f32, tag="ones")

        make_identity(nc, ident_f32[:])
        nc.vector.tensor_copy(ident16[:], ident_f32[:])
        nc.vector.tensor_copy(ident_bf[:], ident_f32[:])
        nc.gpsimd.memset(ones[:], 1.0)
        # Vaug ones column (softmax denominator via the extra matmul column)
        nc.vector.tensor_copy(Vaug[:, :, :, :, D], ones[:])
        # causal mask: with per-chunk 128*m narrowing, ONLY the chunk's own
        # diagonal q-subtile (cols [0:128] of each channel) needs masking —
        # a single [128,2,128] lower-triangle multiplier (DVE, 2-byte fast)
        maskD = const.tile([128, 2, 128], bf16, tag="maskD")
        nc.gpsimd.memset(maskD[:], 1.0)
        nc.gpsimd.affine_select(
            maskD[:],
            maskD[:],
            pattern=[[0, 2], [1, 128]],
            compare_op=ALU.is_ge,
            fill=0.0,
            base=0,
            channel_multiplier=-1,
        )

        tiles = {}

        def qkv_steps(nt):
            """Emit qkv + V-transpose for token-512-tile nt in small steps so
            the caller can interleave them into the attention stream."""
            xt = xp.tile([128, KO, 512], f16, tag="xt")
            if nt == 0:
                # first x half-tile on SP; bias + weights in parallel on the
                # ACT hwdge queue (ACT is idle through the prologue); bias
                # first — the first Pool bias op gates the qkv PSUM rotation
                nc.scalar.dma_start(
                    Wsb[:, 0:4, ds(0, 128)], W_r[:, 0:4, ds(0, 128)]
                )
                nc.scalar.dma_start(bsb[:], bqkv[:])
                nc.sync.dma_start(
                    xt[:, 0:2, :], xT_r[:, 0:2, ds(nt * 512, 512)]
                )
                nc.scalar.dma_start(
                    Wsb[:, 4:KO, ds(0, 128)], W_r[:, 4:KO, ds(0, 128)]
                )
                nc.sync.dma_start(
                    xt[:, 2:4, :], xT_r[:, 2:4, ds(nt * 512, 512)]
                )
                nc.scalar.dma_start(
                    Wsb[:, :, ds(128, 128)], W_r[:, :, ds(128, 128)]
                )
                nc.sync.dma_start(
                    xt[:, 4:6, :], xT_r[:, 4:6, ds(nt * 512, 512)]
                )
                nc.scalar.dma_start(
                    Wsb[:, :, ds(256, 128)], W_r[:, :, ds(256, 128)]
                )
                nc.sync.dma_start(
                    xt[:, 6:KO, :], xT_r[:, 6:KO, ds(nt * 512, 512)]
                )
                for cg in range(3, NCG):
                    nc.scalar.dma_start(
                        Wsb[:, :, ds(cg * 128, 128)],
                        W_r[:, :, ds(cg * 128, 128)],
                    )
            else:
                nc.sync.dma_start(xt[:], xT_r[:, :, ds(nt * 512, 512)])
                if nt == 1:
                    nc.scalar.dma_start(Wp_sb[:], Wp_r[:])
            QTn = qtp.tile([128, 4, 512], f16, tag="qt", name=f"QT{nt}")
            VTn = vtp.tile([128, 4, 512], bf16, tag="vt", name=f"VT{nt}")
            tiles[nt] = QTn
            yield 0
            for cg in range(NCG):
                kind, g = cg // 4, cg % 4
                ps = aux_ps.tile([128, 512], f32, tag="aux")
                for ko in range(KO):
                    nc.tensor.matmul(
                        ps[:],
                        lhsT=Wsb[:, ko, ds(cg * 128, 128)],
                        rhs=xt[:, ko],
                        start=(ko == 0),
                        stop=(ko == KO - 1),
                    )
                    if ko % 2 == 1:
                        yield 427
                bias = bsb[:, ds(cg, 1)]
                if kind == 0:
                    dst = QTn[:, g, :]  # q pre-scaled by 1/8 host-side
                elif kind == 1:
                    dst = KT[:, g, ds(nt * 512, 512)]
                else:
                    dst = VTn[:, g, :]
                nc.vector.tensor_scalar_add(dst, ps[:], bias)
                yield 0
                if kind == 2:
                    # V token-major: one [128,128] transpose covers both
                    # heads of group g for each token-128 chunk
                    for tt in range(4):
                        kcg = nt * 4 + tt
                        tp = aux_ps.tile([128, 2, D], bf16, tag="aux")
                        nc.tensor.transpose(
                            tp[:], VTn[:, g, ds(tt * 128, 128)], ident_bf[:]
                        )
                        nc.vector.tensor_copy(Vaug[:, kcg, g, :, 0:D], tp[:])
                        yield 60

        def proj_steps(js_list):
            """y^T via PE transpose + projection partial for each global
            128-q-subtile in js_list."""
            for js in js_list:
                yTt = ytp.tile([128, 4, 128], f16, tag="yT")
                for hp in range(4):
                    tp = aux_ps.tile([128, 128], f16, tag="aux")
                    nc.tensor.transpose(
                        tp[:], yns[js // 4][:, js % 4, ds(2 * hp, 2), :],
                        ident16[:],
                    )
                    nc.vector.tensor_copy(yTt[:, hp, :], tp[:])
                    yield 60
                for n in range(2):
                    po = aux_ps.tile([128, 512], f32, tag="aux")
                    for dg in range(4):
                        nc.tensor.matmul(
                            po[:],
                            lhsT=yTt[:, dg, :],
                            rhs=Wp_sb[:, dg, ds(n * 512, 512)],
                            start=(dg == 0),
                            stop=(dg == 3),
                        )
                    ob = obp.tile([128, 512], bf16, tag="ob")
                    nc.vector.tensor_copy(ob[:], po[:])
                    nc.sync.dma_start(
                        out[ds(js * 128, 128), ds(n * 512, 512)], ob[:]
                    )
                    yield 853

        def emit_wv(qt, h, prev, yt):
            # PSUM accumulation state is per-BANK (one written-bitmap): the
            # whole tile must form ONE group — start only on the very first
            # matmul, stop on the very last; first write per address within
            # the group replaces, later ones accumulate.
            p, et = prev
            for c in range(2):
                kc = 2 * p + c
                m = kc - 4 * qt
                offc = max(0, 128 * m)
                for j in range(4):
                    if j < m:
                        continue
                    nc.tensor.matmul(
                        yt[:, j, :],
                        lhsT=et[:, c, ds(j * 128 - offc, 128)],
                        rhs=Vaug[:, kc, h // 2, h % 2, :],
                        start=(kc == 0 and j == 0),
                        stop=(kc == 4 * qt + 3 and j == 3),
                        skip_group_check=True,
                    )

        gen0 = qkv_steps(0)
        for _ in gen0:
            pass

        for qt in range(NT):
            QTn = tiles[qt]
            npairs = 2 * qt + 2
            # pump source: next tile's qkv during phases 0-2 (plus the first
            # projection subtiles late in phase 2); remaining projection
            # during the last (exp-heavy) phase. Rate-paced so the PE filler
            # spreads across the whole phase instead of draining early.
            slots = HPC * (2 * npairs + 2)
            if qt + 1 < NT:
                pumps = [qkv_steps(qt + 1)]
                total_ns = 22000.0
            else:
                pumps = [proj_steps(range(0, 12))]
                total_ns = 23100.0
            budget_ns = total_ns / slots
            acc = [0.0]

            def pump(n):
                acc[0] += n * budget_ns
                while acc[0] > 0.0 and pumps:
                    step = next(pumps[0], None)
                    if step is None:
                        pumps.pop(0)
                    else:
                        acc[0] -= max(step, 60.0)

            def finish_head(h, yt):
                """recip + normalize for head h (after its last wv), plus the
                last q-tile's eager per-head-pair transposes."""
                rec = recp.tile([128, 4], f32, tag="rec")
                nc.vector.reciprocal(rec[:], yt[:, :, D])
                for j in range(4):
                    nc.vector.tensor_scalar_mul(
                        yns[qt][:, j, h, :], yt[:, j, 0:D], rec[:, ds(j, 1)]
                    )
                if qt == NT - 1 and h % 2 == 1:
                    hpd = h // 2
                    for jsi in range(4):
                        tp = aux_ps.tile([128, 128], f16, tag="aux")
                        nc.tensor.transpose(
                            tp[:], yns[qt][:, jsi, ds(2 * hpd, 2), :],
                            ident16[:],
                        )
                        nc.vector.tensor_copy(yTtail[:, jsi, hpd, :], tp[:])

            # flat lag-1 pipeline over (head, chunk-pair) units: the wv of
            # unit i-1 is emitted after unit i's scores+exp, ALSO across head
            # boundaries, so no wv ever waits zero-lag on its own exp
            yts = {}
            prev = None
            for h in range(HPC):
                g2, hp = h % 2, h // 2
                for p in range(npairs):
                    m0 = 2 * p - 4 * qt
                    off = max(0, 128 * m0)
                    w = 512 - off
                    sp = sp_ps.tile([128, 2, 512], f32, tag="s")
                    for c in range(2):
                        kc = 2 * p + c
                        offc = max(0, 128 * (m0 + c))
                        nc.tensor.matmul(
                            sp[:, c, 0 : 512 - offc],
                            lhsT=KT[ds(64 * g2, 64), hp, ds(kc * 128, 128)],
                            rhs=QTn[ds(64 * g2, 64), hp, ds(offc, 512 - offc)],
                            start=True,
                            stop=True,
                        )
                    et = expp.tile([128, 2, 512], bf16, tag="exp")
                    nc.scalar.activation(et[:, :, 0:w], sp[:, :, 0:w], AF.Exp)
                    if m0 >= 0:
                        # causal: each chunk's own-diagonal q-subtile sits at
                        # cols [0:128] of its channel after narrowing
                        nc.vector.tensor_mul(
                            et[:, :, 0:128], et[:, :, 0:128], maskD[:]
                        )
                    pump(1)
                    if prev is not None:
                        ph, pp, pet = prev
                        if pp == 0:
                            yts[ph] = y_ps.tile(
                                [128, 4, D + 1], f32, tag="y",
                                name=f"y{qt}_{ph}",
                            )
                        emit_wv(qt, ph, (pp, pet), yts[ph])
                        if pp == npairs - 1:
                            finish_head(ph, yts[ph])
                        pump(1)
                    prev = (h, p, et)
            ph, pp, pet = prev
            if pp == 0:
                yts[ph] = y_ps.tile(
                    [128, 4, D + 1], f32, tag="y", name=f"y{qt}_{ph}"
                )
            pump(2)
            emit_wv(qt, ph, (pp, pet), yts[ph])
            finish_head(ph, yts[ph])
            # drain this phase's pump sources
            while pumps:
                if next(pumps[0], "done") == "done":
                    pumps.pop(0)

        # final projection for the last q-tile's subtiles (transposes already
        # staged in yTtail by the head-pair-gated path above)
        for jsi in range(4):
            for n in range(2):
                po = aux_ps.tile([128, 512], f32, tag="aux")
                for dg in range(4):
                    nc.tensor.matmul(
                        po[:],
                        lhsT=yTtail[:, jsi, dg, :],
                        rhs=Wp_sb[:, dg, ds(n * 512, 512)],
                        start=(dg == 0),
                        stop=(dg == 3),
                    )
                ob = obp.tile([128, 512], bf16, tag="ob")
                nc.scalar.activation(ob[:], po[:], AF.Identity)
                nc.sync.dma_start(
                    out[ds((12 + jsi) * 128, 128), ds(n * 512, 512)], ob[:]
                )

    return nc


def shard_inputs(x, W_attn, b_attn, W_proj):
    """Per-core input maps: core c = 2*b + g owns batch b, heads 8g..8g+8."""
    E = N_EMBD
    nb, s, _ = x.shape
    in_maps = []
    for c in range(N_CORES):
        b, g = c // 2, c % 2
        lo = 512 * g
        Wq = W_attn[:, lo : lo + 512] * 0.125
        Wk = W_attn[:, E + lo : E + lo + 512]
        Wv = W_attn[:, 2 * E + lo : 2 * E + lo + 512]
        Wqkv = np.concatenate([Wq, Wk, Wv], axis=1).astype(np.float16)
        bq = b_attn[lo : lo + 512] * 0.125
        bk = b_attn[E + lo : E + lo + 512]
        bv = b_attn[2 * E + lo : 2 * E + lo + 512]
        bqkv = (
            np.concatenate([bq, bk, bv]).reshape(12, 128).T.astype(np.float32)
        )
        in_maps.append(
            {
                "xT": np.ascontiguousarray(x[b].T).astype(np.float16),
                "Wqkv": np.ascontiguousarray(Wqkv),
                "bqkv": np.ascontiguousarray(bqkv),
                "Wp": W_proj[lo : lo + 512, :].astype(np.float16),
            }
        )
    return in_maps


_NC_CACHE = {}


def _get_nc(s=S_FULL):
    if s not in _NC_CACHE:
        _NC_CACHE[s] = build_nc(s)
    return _NC_CACHE[s]


def kernel(x, W_attn, b_attn, W_proj, b_proj, _trace=False):
    from concourse.bass_utils import run_bass_kernel_spmd

    nb, s, E = x.shape
    assert E == N_EMBD
    nc = _get_nc(s)
    in_maps = shard_inputs(x, W_attn, b_attn, W_proj)
    res = run_bass_kernel_spmd(nc, in_maps, list(range(N_CORES)), trace=_trace)
    outs = []
    for b in range(nb):
        acc = res.results[2 * b]["out"].astype(np.float32)
        acc += res.results[2 * b + 1]["out"]
        acc += b_proj.astype(np.float32)
        outs.append(acc)
    kernel.last_results = res
    return np.stack(outs, axis=0)


# revision 8
# speedup vs baseline: 1.0073x; 1.0065x over previous
"""Causal self-attention (B=4, S=2048, E=1024, H=16) on 8 trn2 cores — v2.

Sharding: batch x head-octet. Core c owns batch b=c//2 and heads
g=c%2 (heads 8g..8g+7, d-slice cols 512g..512g+512):
  - computes q,k,v for its 8 heads from its batch's x,
  - runs causal attention for those heads,
  - multiplies by its 512-row slice of W_proj producing a PARTIAL [S, E]
    output; the host sums each batch's 2 partials and adds b_proj.

Engine balance (single batch per core, ~490k PE cycles):
  - PE: qkv (pumped into the attention phases as filler), scores [k,q],
    flipped w@V (out [128q, 65] per k-chunk: full M=128 vs 65 in the
    [65,w] orientation), V/y transposes, projection.
  - ACT: exp only, batched over chunk-PAIRS ([128,2,512] psum -> bf16),
    plus the tail output copies.
  - DVE: qkv bias add (psum->SBUF, q pre-scaled 1/8 host-side), causal
    mask multiply, reciprocal + normalize, psum->SBUF transpose copies.
  - Pool/GPSIMD cannot touch PSUM on HW; it only builds constants.
  - PSUM accumulation state is per-BANK: each y tile forms ONE
    start/stop group across all its q-subtile regions.
  - The attention inner loop is a flat lag-1 software pipeline over
    (head, chunk-pair) units, including across head boundaries, with a
    cost-weighted pump of qkv/projection filler between units.
  - All projection + y-transposes are DEFERRED to the last attention
    phase (otherwise ACT-exp-bound, and qkv filler cannot exist there).

dtypes: fp16 for x/W/QT/KT/yn/yT/Wp; bf16 for exp(scores) and V (exp of
unnormalized scores needs bf16 range). PSUM f32; output partial bf16,
summed in f32 on host.
"""

import sys

if "/opt/trn_rl_repo" not in sys.path:
    sys.path.insert(0, "/opt/trn_rl_repo")

import numpy as np

N_EMBD = 1024
N_HEAD = 16
D = 64
N_CORES = 8
B_FULL = 4
S_FULL = 2048
HPC = 8  # heads per core


def _patch_tile(tile):
    """This container's walrus build allows max 1 sem wait per instruction;
    stock Tile can attach several (tail drain, and any instruction whose
    inputs come from 2+ engines/queues). Split extras onto standalone
    single-wait nop carriers on the same engine, emitted just before."""
    if getattr(tile.TileContext, "_drain_split_patched", False):
        return

    orig_commit = tile.TileContext._commit_instruction

    def _commit_instruction(self, inst, lazy_reg_writes=True):
        si = inst.sync_info
        waits = list(si.on_wait) if si is not None and si.on_wait else []
        if len(waits) > 1:
            by_name = {h.name: h for h in self.sems.allocated().values()}
            for w in waits[:-1]:
                h = by_name.get(w.ant_name)
                if h is None:
                    raise RuntimeError(f"wait-split: no handle for {w.ant_name}")
                nop = self.nc.engines[inst.engine].nop(nofuse=True)
                nop.wait_op(h, w.wait_value, _wait_mode_op(w), check=False)
            inst.sync_info.on_wait = [waits[-1]]
        return orig_commit(self, inst, lazy_reg_writes)

    def _wait_mode_op(w):
        m = str(w.wait_mode)
        if "ge" in m:
            return "sem-ge"
        if "eq" in m:
            return "sem-eq"
        raise RuntimeError(f"wait-split: unsupported wait mode {m}")

    tile.TileContext._commit_instruction = _commit_instruction

    def _drain_and_barrier(self, tick_clock, wait_clock):
        nc = self.nc
        drain_inst = nc.sync.drain()
        wait_clock.add_sem_waits(
            drain_inst.ins, tile.ScopedClock({None: tick_clock.global_clock})
        )
        waits = list(drain_inst.ins.sync_info.on_wait or [])
        if len(waits) > 1:
            drain_inst.ins.sync_info.on_wait = [waits[0]]
            by_name = {}
            if self.sems is not None:
                by_name = {h.name: h for h in self.sems.allocated().values()}
            for w in waits[1:]:
                extra = nc.sync.drain()
                h = by_name.get(w.ant_name)
                if h is None:
                    raise RuntimeError(f"drain-split: no handle for {w.ant_name}")
                extra._wait_ge(h, w.wait_value)
        nc.all_engine_barrier()
        assert self.sems is not None
        popped = nc._tile_sem_poison_stack.pop()
        assert popped is self._sem_poison
        nc.clear_and_free_semaphores(list(self.sems.allocated().values()))
        nc.all_engine_barrier()

    tile.TileContext._drain_and_barrier = _drain_and_barrier
    tile.TileContext._drain_split_patched = True


def build_nc(s=S_FULL, num_devices=N_CORES):
    import concourse.bass as bass
    import concourse.mybir as mybir
    import concourse.tile as tile
    from concourse.bass import ds, ts
    from concourse.masks import make_identity

    _patch_tile(tile)

    f32 = mybir.dt.float32
    f16 = mybir.dt.float16
    bf16 = mybir.dt.bfloat16
    AF = mybir.ActivationFunctionType
    ALU = mybir.AluOpType
    E = N_EMBD
    KO = E // 128     # contraction chunks for qkv (8)
    NT = s // 512     # 512-token tiles (4)
    NKC = s // 128    # 128-token k chunks (16)
    NQS = s // 128    # 128-token q subtiles (16)
    NCG = 12          # qkv col groups of 128 (4 q, 4 k, 4 v)

    nc = bass.Bass(
        "TRN2", target_bir_lowering=False, debug=False, num_devices=num_devices
    )
    xT = nc.dram_tensor("xT", [E, s], f16, kind="ExternalInput")
    Wqkv = nc.dram_tensor("Wqkv", [E, NCG * 128], f16, kind="ExternalInput")
    bqkv = nc.dram_tensor("bqkv", [128, NCG], f32, kind="ExternalInput")
    Wp = nc.dram_tensor("Wp", [HPC * D, E], f16, kind="ExternalInput")
    out = nc.dram_tensor("out", [s, E], bf16, kind="ExternalOutput")

    xT_r = xT[:].rearrange("(ko p) t -> p ko t", p=128)
    W_r = Wqkv[:].rearrange("(ko p) c -> p ko c", p=128)
    Wp_r = Wp[:].rearrange("(dg p) e -> p dg e", p=128)

    from contextlib import ExitStack

    with tile.TileContext(nc) as tc, ExitStack() as ctx:
        const = ctx.enter_context(tc.tile_pool(name="const", bufs=1))
        xp = ctx.enter_context(tc.tile_pool(name="xp", bufs=2))
        qtp = ctx.enter_context(tc.tile_pool(name="qtp", bufs=2))
        vtp = ctx.enter_context(tc.tile_pool(name="vtp", bufs=2))
        expp = ctx.enter_context(tc.tile_pool(name="expp", bufs=10))
        ytp = ctx.enter_context(tc.tile_pool(name="ytp", bufs=3))
        recp = ctx.enter_context(tc.tile_pool(name="recp", bufs=3))
        obp = ctx.enter_context(tc.tile_pool(name="obp", bufs=4))
        sp_ps = ctx.enter_context(tc.tile_pool(name="spps", bufs=2, space="PSUM"))
        y_ps = ctx.enter_context(tc.tile_pool(name="yps", bufs=2, space="PSUM"))
        aux_ps = ctx.enter_context(tc.tile_pool(name="auxps", bufs=2, space="PSUM"))

        # ---- constants / persistent ----
        Wsb = const.tile([128, KO, NCG * 128], f16, tag="w")
        bsb = const.tile([128, NCG], f32, tag="b")
        Wp_sb = const.tile([128, 4, E], f16, tag="wp")
        KT = const.tile([128, 4, s], f16, tag="kt")
        Vaug = const.tile([128, NKC, 4, 2, D + 1], bf16, tag="vaug")
        yns = [
            const.tile([128, 4, HPC, D], f16, tag=f"yn{q}", name=f"yn{q}")
            for q in range(NT)
        ]
        # yT staging for the last q-tile's subtiles (filled per head-pair as
        # the final phase's heads complete, shrinking the serial tail)
        yTtail = const.tile([128, 4, 4, 128], f16, tag="yTtail")
        ident_f32 = const.tile([128, 128], f32, tag="id32")
        ident16 = const.tile([128, 128], f16, tag="id16")
        ident_bf = const.tile([128, 128], bf16, tag="idbf")
        ones = const.tile([128, NKC, 4, 2], f32, tag="ones")

        make_identity(nc, ident_f32[:])
        nc.vector.tensor_copy(ident16[:], ident_f32[:])
        nc.vector.tensor_copy(ident_bf[:], ident_f32[:])
        nc.gpsimd.memset(ones[:], 1.0)
        # Vaug ones column (softmax denominator via the extra matmul column)
        nc.vector.tensor_copy(Vaug[:, :, :, :, D], ones[:])
        # causal mask: with per-chunk 128*m narrowing, ONLY the chunk's own
        # diagonal q-subtile (cols [0:128] of each channel) needs masking —
        # a single [128,2,128] lower-triangle multiplier (DVE, 2-byte fast)
        maskD = const.tile([128, 2, 128], bf16, tag="maskD")
        nc.gpsimd.memset(maskD[:], 1.0)
        nc.gpsimd.affine_select(
            maskD[:],
            maskD[:],
            pattern=[[0, 2], [1, 128]],
            compare_op=ALU.is_ge,
            fill=0.0,
            base=0,
            channel_multiplier=-1,
        )

        tiles = {}

        def qkv_steps(nt):
            """Emit qkv + V-transpose for token-512-tile nt in small steps so
            the caller can interleave them into the attention stream."""
            xt = xp.tile([128, KO, 512], f16, tag="xt")
            if nt == 0:
                # first x half-tile on SP; bias + weights in parallel on the
                # ACT hwdge queue (ACT is idle through the prologue); bias
                # first — the first Pool bias op gates the qkv PSUM rotation
                nc.scalar.dma_start(
                    Wsb[:, 0:4, ds(0, 128)], W_r[:, 0:4, ds(0, 128)]
                )
                nc.scalar.dma_start(bsb[:], bqkv[:])
                nc.sync.dma_start(
                    xt[:, 0:2, :], xT_r[:, 0:2, ds(nt * 512, 512)]
                )
                nc.scalar.dma_start(
                    Wsb[:, 4:KO, ds(0, 128)], W_r[:, 4:KO, ds(0, 128)]
                )
                nc.sync.dma_start(
                    xt[:, 2:4, :], xT_r[:, 2:4, ds(nt * 512, 512)]
                )
                nc.scalar.dma_start(
                    Wsb[:, :, ds(128, 128)], W_r[:, :, ds(128, 128)]
                )
                nc.sync.dma_start(
                    xt[:, 4:6, :], xT_r[:, 4:6, ds(nt * 512, 512)]
                )
                nc.scalar.dma_start(
                    Wsb[:, :, ds(256, 128)], W_r[:, :, ds(256, 128)]
                )
                nc.sync.dma_start(
                    xt[:, 6:KO, :], xT_r[:, 6:KO, ds(nt * 512, 512)]
                )
                for cg in range(3, NCG):
                    nc.scalar.dma_start(
                        Wsb[:, :, ds(cg * 128, 128)],
                        W_r[:, :, ds(cg * 128, 128)],
                    )
            else:
                nc.sync.dma_start(xt[:], xT_r[:, :, ds(nt * 512, 512)])
                if nt == 1:
                    nc.scalar.dma_start(Wp_sb[:], Wp_r[:])
            QTn = qtp.tile([128, 4, 512], f16, tag="qt", name=f"QT{nt}")
            VTn = vtp.tile([128, 4, 512], bf16, tag="vt", name=f"VT{nt}")
            tiles[nt] = QTn
            yield 0
            for cg in range(NCG):
                kind, g = cg // 4, cg % 4
                ps = aux_ps.tile([128, 512], f32, tag="aux")
                for ko in range(KO):
                    nc.tensor.matmul(
                        ps[:],
                        lhsT=Wsb[:, ko, ds(cg * 128, 128)],
                        rhs=xt[:, ko],
                        start=(ko == 0),
                        stop=(ko == KO - 1),
                    )
                    if ko % 2 == 1:
                        yield 427
                bias = bsb[:, ds(cg, 1)]
                if kind == 0:
                    dst = QTn[:, g, :]  # q pre-scaled by 1/8 host-side
                elif kind == 1:
                    dst = KT[:, g, ds(nt * 512, 512)]
                else:
                    dst = VTn[:, g, :]
                nc.vector.tensor_scalar_add(dst, ps[:], bias)
                yield 0
                if kind == 2:
                    # V token-major: one [128,128] transpose covers both
                    # heads of group g for each token-128 chunk
                    for tt in range(4):
                        kcg = nt * 4 + tt
                        tp = aux_ps.tile([128, 2, D], bf16, tag="aux")
                        nc.tensor.transpose(
                            tp[:], VTn[:, g, ds(tt * 128, 128)], ident_bf[:]
                        )
                        nc.vector.tensor_copy(Vaug[:, kcg, g, :, 0:D], tp[:])
                        yield 60

        def proj_steps(js_list):
            """y^T via PE transpose + projection partial for each global
            128-q-subtile in js_list."""
            for js in js_list:
                yTt = ytp.tile([128, 4, 128], f16, tag="yT")
                for hp in range(4):
                    tp = aux_ps.tile([128, 128], f16, tag="aux")
                    nc.tensor.transpose(
                        tp[:], yns[js // 4][:, js % 4, ds(2 * hp, 2), :],
                        ident16[:],
                    )
                    nc.vector.tensor_copy(yTt[:, hp, :], tp[:])
                    yield 60
                for n in range(2):
                    po = aux_ps.tile([128, 512], f32, tag="aux")
                    for dg in range(4):
                        nc.tensor.matmul(
                            po[:],
                            lhsT=yTt[:, dg, :],
                            rhs=Wp_sb[:, dg, ds(n * 512, 512)],
                            start=(dg == 0),
                            stop=(dg == 3),
                        )
                    ob = obp.tile([128, 512], bf16, tag="ob")
                    nc.vector.tensor_copy(ob[:], po[:])
                    nc.sync.dma_start(
                        out[ds(js * 128, 128), ds(n * 512, 512)], ob[:]
                    )
                    yield 853

        def emit_wv(qt, h, prev, yt):
            # PSUM accumulation state is per-BANK (one written-bitmap): the
            # whole tile must form ONE group — start only on the very first
            # matmul, stop on the very last; first write per address within
            # the group replaces, later ones accumulate.
            p, et = prev
            for c in range(2):
                kc = 2 * p + c
                m = kc - 4 * qt
                offc = max(0, 128 * m)
                for j in range(4):
                    if j < m:
                        continue
                    nc.tensor.matmul(
                        yt[:, j, :],
                        lhsT=et[:, c, ds(j * 128 - offc, 128)],
                        rhs=Vaug[:, kc, h // 2, h % 2, :],
                        start=(kc == 0 and j == 0),
                        stop=(kc == 4 * qt + 3 and j == 3),
                        skip_group_check=True,
                    )

        gen0 = qkv_steps(0)
        for _ in gen0:
            pass

        for qt in range(NT):
            QTn = tiles[qt]
            npairs = 2 * qt + 2
            # pump source: next tile's qkv during phases 0-2 (plus the first
            # projection subtiles late in phase 2); remaining projection
            # during the last (exp-heavy) phase. Rate-paced so the PE filler
            # spreads across the whole phase instead of draining early.
            slots = HPC * (2 * npairs + 2)
            if qt + 1 < NT:
                pumps = [qkv_steps(qt + 1)]
                total_ns = 22000.0
            else:
                pumps = [proj_steps(range(0, 12))]
                total_ns = 23100.0
            budget_ns = total_ns / slots
            acc = [0.0]

            def pump(n):
                acc[0] += n * budget_ns
                while acc[0] > 0.0 and pumps:
                    step = next(pumps[0], None)
                    if step is None:
                        pumps.pop(0)
                    else:
                        acc[0] -= max(step, 60.0)

            def finish_head(h, yt):
                """recip + normalize for head h (after its last wv), plus the
                last q-tile's eager per-head-pair transposes."""
                rec = recp.tile([128, 4], f32, tag="rec")
                nc.vector.reciprocal(rec[:], yt[:, :, D])
                for j in range(4):
                    nc.vector.tensor_scalar_mul(
                        yns[qt][:, j, h, :], yt[:, j, 0:D], rec[:, ds(j, 1)]
                    )
                if qt == NT - 1 and h % 2 == 1:
                    hpd = h // 2
                    for jsi in range(4):
                        tp = aux_ps.tile([128, 128], f16, tag="aux")
                        nc.tensor.transpose(
                            tp[:], yns[qt][:, jsi, ds(2 * hpd, 2), :],
                            ident16[:],
                        )
                        nc.vector.tensor_copy(yTtail[:, jsi, hpd, :], tp[:])

            # flat lag-1 pipeline over (head, chunk-pair) units: the wv of
            # unit i-1 is emitted after unit i's scores+exp, ALSO across head
            # boundaries, so no wv ever waits zero-lag on its own exp
            yts = {}
            prev = None
            for h in range(HPC):
                g2, hp = h % 2, h // 2
                for p in range(npairs):
                    m0 = 2 * p - 4 * qt
                    off = max(0, 128 * m0)
                    w = 512 - off
                    sp = sp_ps.tile([128, 2, 512], f32, tag="s")
                    for c in range(2):
                        kc = 2 * p + c
                        offc = max(0, 128 * (m0 + c))
                        nc.tensor.matmul(
                            sp[:, c, 0 : 512 - offc],
                            lhsT=KT[ds(64 * g2, 64), hp, ds(kc * 128, 128)],
                            rhs=QTn[ds(64 * g2, 64), hp, ds(offc, 512 - offc)],
                            start=True,
                            stop=True,
                        )
                    et = expp.tile([128, 2, 512], bf16, tag="exp")
                    nc.scalar.activation(et[:, :, 0:w], sp[:, :, 0:w], AF.Exp)
                    if m0 >= 0:
                        # causal: each chunk's own-diagonal q-subtile sits at
                        # cols [0:128] of its channel after narrowing
                        nc.vector.tensor_mul(
                            et[:, :, 0:128], et[:, :, 0:128], maskD[:]
                        )
                    pump(1)
                    if prev is not None:
                        ph, pp, pet = prev
                        if pp == 0:
                            yts[ph] = y_ps.tile(
                                [128, 4, D + 1], f32, tag="y",
                                name=f"y{qt}_{ph}",
                            )
                        emit_wv(qt, ph, (pp, pet), yts[ph])
                        if pp == npairs - 1:
                            finish_head(ph, yts[ph])
                        pump(1)
                    prev = (h, p, et)
            ph, pp, pet = prev
            if pp == 0:
                yts[ph] = y_ps.tile(
                    [128, 4, D + 1], f32, tag="y", name=f"y{qt}_{ph}"
                )
            pump(2)
            emit_wv(qt, ph, (pp, pet), yts[ph])
            finish_head(ph, yts[ph])
            # drain this phase's pump sources
            while pumps:
                if next(pumps[0], "done") == "done":
                    pumps.pop(0)

        # final projection for the last q-tile's subtiles (transposes already
        # staged in yTtail by the head-pair-gated path above)
        for jsi in range(4):
            for n in range(2):
                po = aux_ps.tile([128, 512], f32, tag="aux")
                for dg in range(4):
                    nc.tensor.matmul(
                        po[:],
                        lhsT=yTtail[:, jsi, dg, :],
                        rhs=Wp_sb[:, dg, ds(n * 512, 512)],
                        start=(dg == 0),
                        stop=(dg == 3),
                    )
                ob = obp.tile([128, 512], bf16, tag="ob")
                nc.scalar.activation(ob[:], po[:], AF.Identity)
                nc.sync.dma_start(
                    out[ds((12 + jsi) * 128, 128), ds(n * 512, 512)], ob[:]
                )

    return nc


def shard_inputs(x, W_attn, b_attn, W_proj):
    """Per-core input maps: core c = 2*b + g owns batch b, heads 8g..8g+8."""
    E = N_EMBD
    nb, s, _ = x.shape
    in_maps = []
    for c in range(N_CORES):
        b, g = c // 2, c % 2
        lo = 512 * g
        Wq = W_attn[:, lo : lo + 512] * 0.125
        Wk = W_attn[:, E + lo : E + lo + 512]
        Wv = W_attn[:, 2 * E + lo : 2 * E + lo + 512]
        Wqkv = np.concatenate([Wq, Wk, Wv], axis=1).astype(np.float16)
        bq = b_attn[lo : lo + 512] * 0.125
        bk = b_attn[E + lo : E + lo + 512]
        bv = b_attn[2 * E + lo : 2 * E + lo + 512]
        bqkv = (
            np.concatenate([bq, bk, bv]).reshape(12, 128).T.astype(np.float32)
        )
        in_maps.append(
            {
                "xT": np.ascontiguousarray(x[b].T).astype(np.float16),
                "Wqkv": np.ascontiguousarray(Wqkv),
                "bqkv": np.ascontiguousarray(bqkv),
                "Wp": W_proj[lo : lo + 512, :].astype(np.float16),
            }
        )
    return in_maps


_NC_CACHE = {}


def _get_nc(s=S_FULL):
    if s not in _NC_CACHE:
        _NC_CACHE[s] = build_nc(s)
    return _NC_CACHE[s]


def kernel(x, W_attn, b_attn, W_proj, b_proj, _trace=False):
    from concourse.bass_utils import run_bass_kernel_spmd

    nb, s, E = x.shape
    assert E == N_EMBD
    nc = _get_nc(s)
    in_maps = shard_inputs(x, W_attn, b_attn, W_proj)
    res = run_bass_kernel_spmd(nc, in_maps, list(range(N_CORES)), trace=_trace)
    outs = []
    for b in range(nb):
        acc = res.results[2 * b]["out"].astype(np.float32)
        acc += res.results[2 * b + 1]["out"]
        acc += b_proj.astype(np.float32)
        outs.append(acc)
    kernel.last_results = res
    return np.stack(outs, axis=0)
